# revision 2
# baseline (speedup 1.0000x reference)
"""Trainium2 Bass kernel for nn_CountMeanOfFeatureInCluster.

Computation (one training-mode step of a VQ-codebook "count mean" module):
    assign[b] = argmin_c || x[b] - (m[c] - eps) ||_2        (B=8192, C=7, F=2048)
    counts[c], elem_sums[c] = segment counts / segment sums of per-sample
                              feature-sums, by assignment
    scalar_mean[c] = elem_sums[c] / max(counts[c]*F, 1)
    out = where(counts > 32, 0.1*scalar_mean + 0.9*m, m)    # [7, 2048]

Distance argmin is computed via the expansion
    argmin_c dist2 = argmax_c ( <x_b, m'_c> - ||m'_c||^2 / 2 ),  m' = m - eps
so the heavy work is a [B, F] @ [F, C] inner-product matmul. Data-parallel
over 8 NeuronCores (1024 samples each, codebook replicated):

  per core:  DMA x tiles [128, 2048] -> PE-transpose 128x128 blocks ->
             PSUM->SBUF copy (DVE/ACT alternating) -> PE matmul against the
             host-pre-transposed codebook (8 stationary cols = 7 clusters +
             a ones column that yields per-sample feature sums for free) ->
             K=1 matmul folds in the -||m'||^2/2 bias -> PE-transpose scores
             back to [sample, cluster] -> DVE argmax/one-hot/accumulate ->
             final partition-reduction matmul -> per-core [counts|wsums].

Host combines the 8 tiny partial vectors and applies the EMA update.
"""

import numpy as np

import concourse.bacc as bacc
import concourse.bass as bass
import concourse.mybir as mybir
import concourse.tile as tile
from concourse.alu_op_type import AluOpType
from concourse.bass_utils import run_bass_kernel_spmd

EPS = 1e-6
MOMENTUM = 0.1
C = 7
COUNT_THRESH = 32
B, F = 8192, 2048
NCORES = 8
BC = B // NCORES      # samples per core
GROUP = 512           # samples per PSUM accumulation group
NG = BC // GROUP      # groups per core
NT = GROUP // 128     # 128-sample tiles per group
FCH = F // 128        # feature chunks
F32 = mybir.dt.float32

_cache: dict = {}


def _build_nc():
    nc = bacc.Bacc("TRN2", target_bir_lowering=False, debug=False)
    xs_ap = nc.dram_tensor("xs", [BC, F], F32, kind="ExternalInput").ap()
    # mt[p, c*8+n]: chunk c of the transposed codebook, [128 feat, 8] per chunk
    # (cols 0-6 = m' = m - eps, col 7 = 1.0 for per-sample feature sums)
    mt_ap = nc.dram_tensor("mt", [128, FCH * 8], F32, kind="ExternalInput").ap()
    # consts[0, 0:8] = [-||m'_c||^2/2 (c<7), 0]; consts[0, 8:8+GROUP] = 1.0
    consts_ap = nc.dram_tensor("consts", [1, 8 + GROUP], F32, kind="ExternalInput").ap()
    ident_ap = nc.dram_tensor("ident", [128, 128], F32, kind="ExternalInput").ap()
    out_ap = nc.dram_tensor("partials", [2 * C, 1], F32, kind="ExternalOutput").ap()

    with tile.TileContext(nc) as tc:
        with (
            tc.tile_pool(name="const", bufs=1) as const_pool,
            tc.tile_pool(name="x", bufs=2 * NT) as x_pool,
            tc.tile_pool(name="xt", bufs=3) as xt_pool,
            tc.tile_pool(name="sb", bufs=2) as sb_pool,
            tc.tile_pool(name="acc", bufs=1) as acc_pool,
            tc.tile_pool(name="ps_t", bufs=2, space="PSUM") as ps_t,
            tc.tile_pool(name="ps_ip", bufs=2, space="PSUM") as ps_ip,
            tc.tile_pool(name="ps_v", bufs=2, space="PSUM") as ps_v,
            tc.tile_pool(name="ps_r", bufs=1, space="PSUM") as ps_r,
        ):
            mt_t = const_pool.tile([128, FCH * 8], F32)
            nc.sync.dma_start(mt_t[:], mt_ap[:])
            consts_t = const_pool.tile([1, 8 + GROUP], F32)
            nc.sync.dma_start(consts_t[:], consts_ap[:])
            ident_t = const_pool.tile([128, 128], F32)
            nc.sync.dma_start(ident_t[:], ident_ap[:])

            acc = acc_pool.tile([128, 2 * C], F32)
            nc.vector.memset(acc[:], 0.0)
            ones_red = const_pool.tile([128, 1], F32)
            nc.vector.memset(ones_red[:], 1.0)

            copy_flip = 0
            for g in range(NG):
                xts = []
                for q in range(NT):
                    xt = x_pool.tile([128, F], F32, tag="x")
                    st = g * NT + q
                    nc.sync.dma_start(xt[:], xs_ap[st * 128:(st + 1) * 128, :])
                    xts.append(xt)

                ipps = ps_ip.tile([8, GROUP], F32)
                for fc in range(FCH):
                    tp = ps_t.tile([128, GROUP], F32)
                    for q in range(NT):
                        nc.tensor.transpose(
                            tp[:, q * 128:(q + 1) * 128],
                            xts[q][:, fc * 128:(fc + 1) * 128],
                            ident_t[:],
                        )
                    xT = xt_pool.tile([128, GROUP], F32)
                    if copy_flip % 2 == 0:
                        nc.scalar.copy(xT[:], tp[:])
                    else:
                        nc.vector.tensor_copy(xT[:], tp[:])
                    copy_flip += 1
                    nc.tensor.matmul(
                        ipps[:],
                        lhsT=mt_t[:, fc * 8:(fc + 1) * 8],
                        rhs=xT[:],
                        start=(fc == 0),
                        stop=False,
                    )
                # K=1 outer product adds the per-cluster bias -||m'_c||^2/2
                nc.tensor.matmul(
                    ipps[:],
                    lhsT=consts_t[0:1, 0:8],
                    rhs=consts_t[0:1, 8:8 + GROUP],
                    start=False,
                    stop=True,
                )
                sc = sb_pool.tile([8, GROUP], F32, tag="sc")
                nc.scalar.copy(sc[:], ipps[:])
                for q in range(NT):
                    vps = ps_v.tile([128, 8], F32)
                    nc.tensor.transpose(
                        vps[:], sc[:, q * 128:(q + 1) * 128], ident_t[0:8, 0:8]
                    )
                    # vps: [128 samples, 8] = 7 biased scores + feature-sum
                    mx = sb_pool.tile([128, 1], F32, tag="mx")
                    nc.vector.tensor_reduce(
                        mx[:], vps[:, 0:C], axis=mybir.AxisListType.X, op=AluOpType.max
                    )
                    # counts += (score == rowmax)
                    nc.vector.scalar_tensor_tensor(
                        acc[:, 0:C], vps[:, 0:C], mx[:, 0:1], acc[:, 0:C],
                        op0=AluOpType.is_equal, op1=AluOpType.add,
                    )
                    # wsums += (score == rowmax) * feature_sum
                    whm = sb_pool.tile([128, C], F32, tag="whm")
                    nc.vector.tensor_scalar(
                        whm[:], vps[:, 0:C], mx[:, 0:1], vps[:, C:C + 1],
                        op0=AluOpType.is_equal, op1=AluOpType.mult,
                    )
                    nc.vector.tensor_tensor(
                        acc[:, C:2 * C], acc[:, C:2 * C], whm[:], op=AluOpType.add
                    )

            rps = ps_r.tile([2 * C, 1], F32)
            nc.tensor.matmul(rps[:], lhsT=acc[:, 0:2 * C], rhs=ones_red[:],
                             start=True, stop=True)
            res_sb = sb_pool.tile([2 * C, 1], F32, tag="res")
            nc.vector.tensor_copy(res_sb[:], rps[:])
            nc.sync.dma_start(out_ap[:], res_sb[:])

    nc.compile()
    return nc


def _get_nc():
    if "nc" not in _cache:
        _cache["nc"] = _build_nc()
    return _cache["nc"]


def _host_inputs(running_mean: np.ndarray):
    mp = running_mean.astype(np.float64) - EPS          # [C, F]
    mt_aug = np.zeros((F, 8), dtype=np.float64)
    mt_aug[:, :C] = mp.T
    mt_aug[:, C] = 1.0
    # SBUF chunk layout: mt[p, c*8+n] = mt_aug[c*128+p, n]
    mt = np.ascontiguousarray(
        mt_aug.reshape(FCH, 128, 8).transpose(1, 0, 2).reshape(128, FCH * 8)
    ).astype(np.float32)
    consts = np.zeros((1, 8 + GROUP), dtype=np.float32)
    consts[0, :C] = (-0.5 * (mp * mp).sum(axis=1)).astype(np.float32)
    consts[0, C] = 0.0
    consts[0, 8:] = 1.0
    ident = np.eye(128, dtype=np.float32)
    return mt, consts, ident


def kernel(x: np.ndarray, running_mean: np.ndarray) -> np.ndarray:
    x = np.asarray(x, dtype=np.float32)
    running_mean = np.asarray(running_mean, dtype=np.float32)
    nc = _get_nc()
    mt, consts, ident = _host_inputs(running_mean)
    in_maps = [
        {
            "xs": np.ascontiguousarray(x[i * BC:(i + 1) * BC]),
            "mt": mt,
            "consts": consts,
            "ident": ident,
        }
        for i in range(NCORES)
    ]
    res = run_bass_kernel_spmd(nc, in_maps, core_ids=list(range(NCORES)))
    counts = np.zeros(C, dtype=np.float32)
    wsums = np.zeros(C, dtype=np.float32)
    for r in res.results:
        p = r["partials"].reshape(2 * C)
        counts += p[:C]
        wsums += p[C:]
    scalar_mean = wsums / np.maximum(counts * np.float32(F), np.float32(1.0))
    update = (np.float32(MOMENTUM) * scalar_mean)[:, None] + np.float32(
        1.0 - MOMENTUM
    ) * running_mean
    out = np.where((counts > COUNT_THRESH)[:, None], update, running_mean)
    return out.astype(np.float32)


# revision 4
# speedup vs baseline: 2.3134x; 2.3134x over previous
"""Trainium2 Bass kernel for nn_CountMeanOfFeatureInCluster.

Computation (one training-mode step of a VQ-codebook "count mean" module):
    assign[b] = argmin_c || x[b] - (m[c] - eps) ||_2        (B=8192, C=7, F=2048)
    counts[c], elem_sums[c] = segment counts / segment sums of per-sample
                              feature-sums, by assignment
    scalar_mean[c] = elem_sums[c] / max(counts[c]*F, 1)
    out = where(counts > 32, 0.1*scalar_mean + 0.9*m, m)    # [7, 2048]

Distance argmin via the expansion
    argmin_c dist2 = argmax_c ( <x_b, m'_c> - ||m'_c||^2 / 2 ),  m' = m - eps
so the heavy work is a [B, F] @ [F, C] inner-product matmul. Data-parallel
over 8 NeuronCores (1024 samples each, codebook replicated):

  per core:  SWDGE cast-DMA x tiles f32->bf16 [128, 2048] -> PE-transpose
             128x128 bf16 blocks -> PSUM->SBUF copy (DVE/ACT alternating) ->
             PE matmul (bf16) against the host-pre-transposed codebook
             (8 stationary cols = 7 clusters + a ones column that yields
             per-sample feature sums for free) -> bias-add the -||m'||^2/2
             term during the f32 score copy (per-partition scalar) ->
             PE-transpose scores back to [sample, cluster] -> DVE argmax/
             one-hot/accumulate -> final partition-reduction matmul ->
             per-core [counts|wsums].

bf16 is safe here: scores only pick an argmax whose typical cluster gap is
O(100) in dist^2 units, and the output is 0.1 * (sums / (counts*2048)), so
per-sample rounding shrinks by ~2.4e6 before reaching the output.

Host combines the 8 tiny partial vectors and applies the EMA update.
"""

import numpy as np

import concourse.bacc as bacc
import concourse.bass as bass
import concourse.mybir as mybir
import concourse.tile as tile
from concourse.alu_op_type import AluOpType
from concourse.bass_utils import run_bass_kernel_spmd

EPS = 1e-6
MOMENTUM = 0.1
C = 7
COUNT_THRESH = 32
B, F = 8192, 2048
NCORES = 8
BC = B // NCORES      # samples per core
GROUP = 512           # samples per PSUM accumulation group
NG = BC // GROUP      # groups per core
NT = GROUP // 128     # 128-sample tiles per group
FCH = F // 128        # feature chunks
F32 = mybir.dt.float32
BF16 = mybir.dt.bfloat16

_cache: dict = {}


def _build_nc():
    nc = bacc.Bacc("TRN2", target_bir_lowering=False, debug=False)
    xs_ap = nc.dram_tensor("xs", [BC, F], F32, kind="ExternalInput").ap()
    # mt[p, c*8+n]: chunk c of the transposed codebook (bf16), [128 feat, 8]
    # per chunk (cols 0-6 = m' = m - eps, col 7 = 1.0 for feature sums)
    mt_ap = nc.dram_tensor("mt", [128, FCH * 8], BF16, kind="ExternalInput").ap()
    # hb[c, 0] = -||m'_c||^2/2 for c<7, hb[7, 0] = 0 (keeps the feature-sum row)
    hb_ap = nc.dram_tensor("hb", [8, 1], F32, kind="ExternalInput").ap()
    identb_ap = nc.dram_tensor("identb", [128, 128], BF16, kind="ExternalInput").ap()
    id8_ap = nc.dram_tensor("id8", [8, 8], F32, kind="ExternalInput").ap()
    out_ap = nc.dram_tensor("partials", [2 * C, 1], F32, kind="ExternalOutput").ap()

    with tile.TileContext(nc) as tc:
        with (
            tc.tile_pool(name="const", bufs=1) as const_pool,
            tc.tile_pool(name="x", bufs=2 * NT) as x_pool,
            tc.tile_pool(name="xt", bufs=4) as xt_pool,
            tc.tile_pool(name="sb", bufs=2) as sb_pool,
            tc.tile_pool(name="acc", bufs=1) as acc_pool,
            tc.tile_pool(name="ps_t", bufs=3, space="PSUM") as ps_t,
            tc.tile_pool(name="ps_ip", bufs=2, space="PSUM") as ps_ip,
            tc.tile_pool(name="ps_v", bufs=2, space="PSUM") as ps_v,
            tc.tile_pool(name="ps_r", bufs=1, space="PSUM") as ps_r,
        ):
            mt_t = const_pool.tile([128, FCH * 8], BF16)
            nc.sync.dma_start(mt_t[:], mt_ap[:])
            hb_t = const_pool.tile([8, 1], F32)
            nc.sync.dma_start(hb_t[:], hb_ap[:])
            identb_t = const_pool.tile([128, 128], BF16)
            nc.sync.dma_start(identb_t[:], identb_ap[:])
            id8_t = const_pool.tile([8, 8], F32)
            nc.sync.dma_start(id8_t[:], id8_ap[:])

            acc = acc_pool.tile([128, 2 * C], F32)
            nc.vector.memset(acc[:], 0.0)
            ones_red = const_pool.tile([128, 1], F32)
            nc.vector.memset(ones_red[:], 1.0)

            copy_flip = 0
            for g in range(NG):
                xts = []
                for q in range(NT):
                    xt = x_pool.tile([128, F], BF16, tag="x")
                    st = g * NT + q
                    # SWDGE cast-DMA: f32 DRAM -> bf16 SBUF
                    nc.gpsimd.dma_start(xt[:], xs_ap[st * 128:(st + 1) * 128, :])
                    xts.append(xt)

                ipps = ps_ip.tile([8, GROUP], F32)
                for fc in range(FCH):
                    tp = ps_t.tile([128, GROUP], BF16)
                    for q in range(NT):
                        nc.tensor.transpose(
                            tp[:, q * 128:(q + 1) * 128],
                            xts[q][:, fc * 128:(fc + 1) * 128],
                            identb_t[:],
                        )
                    xT = xt_pool.tile([128, GROUP], BF16)
                    if copy_flip % 2 == 0:
                        nc.scalar.copy(xT[:], tp[:])
                    else:
                        nc.vector.tensor_copy(xT[:], tp[:])
                    copy_flip += 1
                    nc.tensor.matmul(
                        ipps[:],
                        lhsT=mt_t[:, fc * 8:(fc + 1) * 8],
                        rhs=xT[:],
                        start=(fc == 0),
                        stop=(fc == FCH - 1),
                    )
                # biased scores: sc[c, s] = ip[c, s] - ||m'_c||^2/2   (f32)
                sc = sb_pool.tile([8, GROUP], F32, tag="sc")
                nc.vector.tensor_scalar(
                    sc[:], ipps[:], hb_t[0:8, 0:1], None, op0=AluOpType.add
                )
                for q in range(NT):
                    vps = ps_v.tile([128, 8], F32)
                    nc.tensor.transpose(
                        vps[:], sc[:, q * 128:(q + 1) * 128], id8_t[:]
                    )
                    # vps: [128 samples, 8] = 7 biased scores + feature-sum
                    mx = sb_pool.tile([128, 1], F32, tag="mx")
                    nc.vector.tensor_reduce(
                        mx[:], vps[:, 0:C], axis=mybir.AxisListType.X, op=AluOpType.max
                    )
                    # counts += (score == rowmax)
                    nc.vector.scalar_tensor_tensor(
                        acc[:, 0:C], vps[:, 0:C], mx[:, 0:1], acc[:, 0:C],
                        op0=AluOpType.is_equal, op1=AluOpType.add,
                    )
                    # wsums += (score == rowmax) * feature_sum
                    whm = sb_pool.tile([128, C], F32, tag="whm")
                    nc.vector.tensor_scalar(
                        whm[:], vps[:, 0:C], mx[:, 0:1], vps[:, C:C + 1],
                        op0=AluOpType.is_equal, op1=AluOpType.mult,
                    )
                    nc.vector.tensor_tensor(
                        acc[:, C:2 * C], acc[:, C:2 * C], whm[:], op=AluOpType.add
                    )

            rps = ps_r.tile([2 * C, 1], F32)
            nc.tensor.matmul(rps[:], lhsT=acc[:, 0:2 * C], rhs=ones_red[:],
                             start=True, stop=True)
            res_sb = sb_pool.tile([2 * C, 1], F32, tag="res")
            nc.vector.tensor_copy(res_sb[:], rps[:])
            nc.sync.dma_start(out_ap[:], res_sb[:])

    nc.compile()
    return nc


def _get_nc():
    if "nc" not in _cache:
        _cache["nc"] = _build_nc()
    return _cache["nc"]


def _host_inputs(running_mean: np.ndarray):
    mp = running_mean.astype(np.float64) - EPS          # [C, F]
    mt_aug = np.zeros((F, 8), dtype=np.float64)
    mt_aug[:, :C] = mp.T
    mt_aug[:, C] = 1.0
    # SBUF chunk layout: mt[p, c*8+n] = mt_aug[c*128+p, n]
    mt = np.ascontiguousarray(
        mt_aug.reshape(FCH, 128, 8).transpose(1, 0, 2).reshape(128, FCH * 8)
    ).astype(_bf16_np())
    hb = np.zeros((8, 1), dtype=np.float32)
    # bias must match what the PE actually multiplies: the bf16-rounded m'
    mpb = mt_aug[:, :C].astype(_bf16_np()).astype(np.float64)
    hb[:C, 0] = (-0.5 * (mpb * mpb).sum(axis=0)).astype(np.float32)
    identb = np.eye(128).astype(_bf16_np())
    id8 = np.eye(8, dtype=np.float32)
    return mt, hb, identb, id8


def _bf16_np():
    import ml_dtypes

    return np.dtype(ml_dtypes.bfloat16)


def kernel(x: np.ndarray, running_mean: np.ndarray) -> np.ndarray:
    x = np.asarray(x, dtype=np.float32)
    running_mean = np.asarray(running_mean, dtype=np.float32)
    nc = _get_nc()
    mt, hb, identb, id8 = _host_inputs(running_mean)
    in_maps = [
        {
            "xs": np.ascontiguousarray(x[i * BC:(i + 1) * BC]),
            "mt": mt,
            "hb": hb,
            "identb": identb,
            "id8": id8,
        }
        for i in range(NCORES)
    ]
    res = run_bass_kernel_spmd(nc, in_maps, core_ids=list(range(NCORES)))
    counts = np.zeros(C, dtype=np.float32)
    wsums = np.zeros(C, dtype=np.float32)
    for r in res.results:
        p = r["partials"].reshape(2 * C)
        counts += p[:C]
        wsums += p[C:]
    scalar_mean = wsums / np.maximum(counts * np.float32(F), np.float32(1.0))
    update = (np.float32(MOMENTUM) * scalar_mean)[:, None] + np.float32(
        1.0 - MOMENTUM
    ) * running_mean
    out = np.where((counts > COUNT_THRESH)[:, None], update, running_mean)
    return out.astype(np.float32)


# revision 5
# speedup vs baseline: 2.4162x; 1.0445x over previous
"""Trainium2 Bass kernel for nn_CountMeanOfFeatureInCluster.

Computation (one training-mode step of a VQ-codebook "count mean" module):
    assign[b] = argmin_c || x[b] - (m[c] - eps) ||_2        (B=8192, C=7, F=2048)
    counts[c], elem_sums[c] = segment counts / segment sums of per-sample
                              feature-sums, by assignment
    scalar_mean[c] = elem_sums[c] / max(counts[c]*F, 1)
    out = where(counts > 32, 0.1*scalar_mean + 0.9*m, m)    # [7, 2048]

Distance argmin via the expansion
    argmin_c dist2 = argmax_c ( <x_b, m'_c> - ||m'_c||^2 / 2 ),  m' = m - eps
so the heavy work is a [B, F] @ [F, C] inner-product matmul. Data-parallel
over 8 NeuronCores (1024 samples each, codebook replicated):

  per core:  SWDGE cast-DMA x f32->bf16 (batched tiles) -> PE-transpose
             128x128 bf16 blocks -> PSUM->SBUF copy (DVE/ACT alternating) ->
             PE matmul (bf16) against the host-pre-transposed codebook
             (8 stationary cols = 7 clusters + a ones column that yields
             per-sample feature sums for free) -> ACT Identity+bias adds
             -||m'||^2/2 during the f32 score copy -> PE-transpose scores
             back to [sample, cluster] -> batched DVE argmax/one-hot/
             accumulate (broadcast APs) -> final partition-reduction matmul
             -> per-core [counts|wsums] partials.

bf16 is safe here: scores only pick an argmax whose typical cluster gap is
O(100) in dist^2 units, and the output is 0.1 * (sums / (counts*2048)), so
per-sample rounding shrinks by ~2.4e6 before reaching the output.

Host combines the 8 tiny partial vectors and applies the EMA update.
"""

import numpy as np

import concourse.bacc as bacc
import concourse.bass as bass
import concourse.mybir as mybir
import concourse.tile as tile
from concourse.alu_op_type import AluOpType
from concourse.bass_utils import run_bass_kernel_spmd

EPS = 1e-6
MOMENTUM = 0.1
C = 7
COUNT_THRESH = 32
B, F = 8192, 2048
NCORES = 8
BC = B // NCORES      # samples per core
GROUP = 512           # samples per PSUM accumulation group
NG = BC // GROUP      # groups per core
NT = GROUP // 128     # 128-sample tiles per group
FCH = F // 128        # feature chunks
DMA_BATCH = 2         # 128-sample tiles per cast-DMA
F32 = mybir.dt.float32
BF16 = mybir.dt.bfloat16

_cache: dict = {}


def _build_nc():
    nc = bacc.Bacc("TRN2", target_bir_lowering=False, debug=False)
    xs_ap = nc.dram_tensor("xs", [BC, F], F32, kind="ExternalInput").ap()
    # mt[p, c*8+n]: chunk c of the transposed codebook (bf16), [128 feat, 8]
    # per chunk (cols 0-6 = m' = m - eps, col 7 = 1.0 for feature sums)
    mt_ap = nc.dram_tensor("mt", [128, FCH * 8], BF16, kind="ExternalInput").ap()
    # hb[c, 0] = -||m'_c||^2/2 for c<7, hb[7, 0] = 0 (keeps the feature-sum row)
    hb_ap = nc.dram_tensor("hb", [8, 1], F32, kind="ExternalInput").ap()
    identb_ap = nc.dram_tensor("identb", [128, 128], BF16, kind="ExternalInput").ap()
    id8_ap = nc.dram_tensor("id8", [8, 8], F32, kind="ExternalInput").ap()
    out_ap = nc.dram_tensor("partials", [2 * NT * C, 1], F32, kind="ExternalOutput").ap()

    n_dma = BC // (128 * DMA_BATCH)
    xs_t = xs_ap.rearrange("(d q p) f -> d p q f", p=128, q=DMA_BATCH)

    with tile.TileContext(nc) as tc:
        with (
            tc.tile_pool(name="const", bufs=1) as const_pool,
            tc.tile_pool(name="x", bufs=2 * NT // DMA_BATCH) as x_pool,
            tc.tile_pool(name="xt", bufs=4) as xt_pool,
            tc.tile_pool(name="sb", bufs=2) as sb_pool,
            tc.tile_pool(name="acc", bufs=1) as acc_pool,
            tc.tile_pool(name="ps_t", bufs=3, space="PSUM") as ps_t,
            tc.tile_pool(name="ps_ip", bufs=2, space="PSUM") as ps_ip,
            tc.tile_pool(name="ps_v", bufs=2, space="PSUM") as ps_v,
            tc.tile_pool(name="ps_r", bufs=1, space="PSUM") as ps_r,
        ):
            mt_t = const_pool.tile([128, FCH * 8], BF16)
            nc.sync.dma_start(mt_t[:], mt_ap[:])
            hb_t = const_pool.tile([8, 1], F32)
            nc.sync.dma_start(hb_t[:], hb_ap[:])
            identb_t = const_pool.tile([128, 128], BF16)
            nc.sync.dma_start(identb_t[:], identb_ap[:])
            id8_t = const_pool.tile([8, 8], F32)
            nc.sync.dma_start(id8_t[:], id8_ap[:])

            acc = acc_pool.tile([128, 2, NT, C], F32)
            nc.vector.memset(acc[:], 0.0)
            ones_red = const_pool.tile([128, 1], F32)
            nc.vector.memset(ones_red[:], 1.0)

            # prefetch all cast-DMAs up front (SWDGE: f32 DRAM -> bf16 SBUF)
            xds = []
            for d in range(n_dma):
                xd = x_pool.tile([128, DMA_BATCH, F], BF16, tag="x")
                nc.gpsimd.dma_start(xd[:], xs_t[d])
                xds.append(xd)

            def xblock(st, fc):
                d, q = divmod(st, DMA_BATCH)
                return xds[d][:, q, fc * 128:(fc + 1) * 128]

            copy_flip = 0
            for g in range(NG):
                ipps = ps_ip.tile([8, GROUP], F32)
                for fc in range(FCH):
                    tp = ps_t.tile([128, GROUP], BF16)
                    for q in range(NT):
                        nc.tensor.transpose(
                            tp[:, q * 128:(q + 1) * 128],
                            xblock(g * NT + q, fc),
                            identb_t[:],
                        )
                    xT = xt_pool.tile([128, GROUP], BF16)
                    if copy_flip % 2 == 0:
                        nc.scalar.copy(xT[:], tp[:])
                    else:
                        nc.vector.tensor_copy(xT[:], tp[:])
                    copy_flip += 1
                    nc.tensor.matmul(
                        ipps[:],
                        lhsT=mt_t[:, fc * 8:(fc + 1) * 8],
                        rhs=xT[:],
                        start=(fc == 0),
                        stop=(fc == FCH - 1),
                    )
                # biased scores on ACT: sc[c, s] = ip[c, s] - ||m'_c||^2/2
                sc = sb_pool.tile([8, GROUP], F32, tag="sc")
                nc.scalar.activation(
                    sc[:], ipps[:], mybir.ActivationFunctionType.Identity,
                    bias=hb_t[0:8, 0:1],
                )
                vps = ps_v.tile([128, NT, 8], F32)
                for q in range(NT):
                    nc.tensor.transpose(
                        vps[:, q, :], sc[:, q * 128:(q + 1) * 128], id8_t[:]
                    )
                # vps: [128 samples, q, 8] = 7 biased scores + feature-sum
                mxg = sb_pool.tile([128, NT], F32, tag="mx")
                nc.vector.tensor_reduce(
                    mxg[:], vps[:, :, 0:C], axis=mybir.AxisListType.X,
                    op=AluOpType.max,
                )
                ohg = sb_pool.tile([128, NT, C], F32, tag="oh")
                nc.vector.tensor_tensor(
                    ohg[:], vps[:, :, 0:C], mxg[:].broadcast_to([128, NT, C]),
                    op=AluOpType.is_equal,
                )
                nc.vector.tensor_tensor(
                    acc[:, 0, :, :], acc[:, 0, :, :], ohg[:], op=AluOpType.add
                )
                whg = sb_pool.tile([128, NT, C], F32, tag="whm")
                nc.vector.tensor_tensor(
                    whg[:], ohg[:], vps[:, :, C:C + 1].broadcast_to([128, NT, C]),
                    op=AluOpType.mult,
                )
                nc.vector.tensor_tensor(
                    acc[:, 1, :, :], acc[:, 1, :, :], whg[:], op=AluOpType.add
                )

            rps = ps_r.tile([2 * NT * C, 1], F32)
            nc.tensor.matmul(
                rps[:], lhsT=acc[:].rearrange("p a q c -> p (a q c)"),
                rhs=ones_red[:], start=True, stop=True,
            )
            res_sb = sb_pool.tile([2 * NT * C, 1], F32, tag="res")
            nc.vector.tensor_copy(res_sb[:], rps[:])
            nc.sync.dma_start(out_ap[:], res_sb[:])

    nc.compile()
    return nc


def _get_nc():
    if "nc" not in _cache:
        _cache["nc"] = _build_nc()
    return _cache["nc"]


def _bf16_np():
    import ml_dtypes

    return np.dtype(ml_dtypes.bfloat16)


def _host_inputs(running_mean: np.ndarray):
    mp = running_mean.astype(np.float64) - EPS          # [C, F]
    mt_aug = np.zeros((F, 8), dtype=np.float64)
    mt_aug[:, :C] = mp.T
    mt_aug[:, C] = 1.0
    # SBUF chunk layout: mt[p, c*8+n] = mt_aug[c*128+p, n]
    mt = np.ascontiguousarray(
        mt_aug.reshape(FCH, 128, 8).transpose(1, 0, 2).reshape(128, FCH * 8)
    ).astype(_bf16_np())
    hb = np.zeros((8, 1), dtype=np.float32)
    # bias matches what the PE actually multiplies: the bf16-rounded m'
    mpb = mt_aug[:, :C].astype(_bf16_np()).astype(np.float64)
    hb[:C, 0] = (-0.5 * (mpb * mpb).sum(axis=0)).astype(np.float32)
    identb = np.eye(128).astype(_bf16_np())
    id8 = np.eye(8, dtype=np.float32)
    return mt, hb, identb, id8


def kernel(x: np.ndarray, running_mean: np.ndarray) -> np.ndarray:
    x = np.asarray(x, dtype=np.float32)
    running_mean = np.asarray(running_mean, dtype=np.float32)
    nc = _get_nc()
    mt, hb, identb, id8 = _host_inputs(running_mean)
    in_maps = [
        {
            "xs": np.ascontiguousarray(x[i * BC:(i + 1) * BC]),
            "mt": mt,
            "hb": hb,
            "identb": identb,
            "id8": id8,
        }
        for i in range(NCORES)
    ]
    res = run_bass_kernel_spmd(nc, in_maps, core_ids=list(range(NCORES)))
    counts = np.zeros(C, dtype=np.float32)
    wsums = np.zeros(C, dtype=np.float32)
    for r in res.results:
        p = r["partials"].reshape(2, NT, C)
        counts += p[0].sum(axis=0)
        wsums += p[1].sum(axis=0)
    scalar_mean = wsums / np.maximum(counts * np.float32(F), np.float32(1.0))
    update = (np.float32(MOMENTUM) * scalar_mean)[:, None] + np.float32(
        1.0 - MOMENTUM
    ) * running_mean
    out = np.where((counts > COUNT_THRESH)[:, None], update, running_mean)
    return out.astype(np.float32)


# revision 11
# speedup vs baseline: 2.5513x; 1.0559x over previous
"""Trainium2 Bass kernel for nn_CountMeanOfFeatureInCluster.

Computation (one training-mode step of a VQ-codebook "count mean" module):
    assign[b] = argmin_c || x[b] - (m[c] - eps) ||_2        (B=8192, C=7, F=2048)
    counts[c], elem_sums[c] = segment counts / segment sums of per-sample
                              feature-sums, by assignment
    scalar_mean[c] = elem_sums[c] / max(counts[c]*F, 1)
    out = where(counts > 32, 0.1*scalar_mean + 0.9*m, m)    # [7, 2048]

Distance argmin via the expansion
    argmin_c dist2 = argmax_c ( <x_b, m'_c> - ||m'_c||^2 / 2 ),  m' = m - eps
so the heavy work is a [B, F] @ [F, C] inner-product matmul. Data-parallel
over 8 NeuronCores (1024 samples each, codebook replicated):

  per core:  SWDGE cast-DMA x f32->bf16 (batched tiles) -> PE-transpose
             128x128 bf16 blocks -> PSUM->SBUF copy (DVE/ACT alternating) ->
             PE matmul (bf16) against the host-pre-transposed codebook
             (8 stationary cols = 7 clusters + a ones column that yields
             per-sample feature sums for free) -> ACT Identity+bias adds
             -||m'||^2/2 during the f32 score copy -> PE-transpose scores
             back to [sample, cluster] -> batched DVE argmax/one-hot/
             accumulate (broadcast APs) -> final partition-reduction matmul
             -> per-core [counts|wsums] partials.

bf16 is safe here: scores only pick an argmax whose typical cluster gap is
O(100) in dist^2 units, and the output is 0.1 * (sums / (counts*2048)), so
per-sample rounding shrinks by ~2.4e6 before reaching the output.

Host combines the 8 tiny partial vectors and applies the EMA update.
"""

import numpy as np

import concourse.bacc as bacc
import concourse.bass as bass
import concourse.mybir as mybir
import concourse.tile as tile
from concourse.alu_op_type import AluOpType
from concourse.bass_utils import run_bass_kernel_spmd

EPS = 1e-6
MOMENTUM = 0.1
C = 7
COUNT_THRESH = 32
B, F = 8192, 2048
NCORES = 8
BC = B // NCORES      # samples per core
GROUP = 512           # samples per PSUM accumulation group
NG = BC // GROUP      # groups per core
NT = GROUP // 128     # 128-sample tiles per group
FCH = F // 128        # feature chunks
DMA_BATCH = 1         # 128-sample tiles per cast-DMA
FC_PER_COPY = 2       # feature chunks per PSUM->SBUF copy
COPY_PATTERN = (0, 1, 0, 1, 0, 0, 1, 0)  # 0 = DVE, 1 = ACT (5:3 toward DVE)
XBAR = False          # PE transposes keep the PE warm; xbar serializes DMA
F32 = mybir.dt.float32
BF16 = mybir.dt.bfloat16

_cache: dict = {}


def _build_nc():
    nc = bacc.Bacc("TRN2", target_bir_lowering=False, debug=False)
    xs_ap = nc.dram_tensor("xs", [BC, F], F32, kind="ExternalInput").ap()
    # mt[p, c*8+n]: chunk c of the transposed codebook (bf16), [128 feat, 8]
    # per chunk (cols 0-6 = m' = m - eps, col 7 = 1.0 for feature sums)
    mt_ap = nc.dram_tensor("mt", [128, FCH * 8], BF16, kind="ExternalInput").ap()
    # hb[c, 0] = -||m'_c||^2/2 for c<7, hb[7, 0] = 0 (keeps the feature-sum row)
    hb_ap = nc.dram_tensor("hb", [8, 1], F32, kind="ExternalInput").ap()
    identb_ap = nc.dram_tensor("identb", [128, 128], BF16, kind="ExternalInput").ap()
    id8_ap = nc.dram_tensor("id8", [8, 8], F32, kind="ExternalInput").ap()
    out_ap = nc.dram_tensor("partials", [2 * NT * C, 1], F32, kind="ExternalOutput").ap()

    n_dma = BC // (128 * DMA_BATCH)
    xs_t = xs_ap.rearrange("(d q p) f -> d p q f", p=128, q=DMA_BATCH)

    with tile.TileContext(nc) as tc:
        with (
            tc.tile_pool(name="const", bufs=1) as const_pool,
            tc.tile_pool(name="x", bufs=2 * NT // DMA_BATCH) as x_pool,
            tc.tile_pool(name="xt", bufs=(2 if XBAR else 4)) as xt_pool,
            tc.tile_pool(name="sb", bufs=2) as sb_pool,
            tc.tile_pool(name="acc", bufs=1) as acc_pool,
            tc.tile_pool(name="ps_t", bufs=(1 if XBAR else 3), space="PSUM") as ps_t,
            tc.tile_pool(name="ps_ip", bufs=2, space="PSUM") as ps_ip,
            tc.tile_pool(name="ps_v", bufs=2, space="PSUM") as ps_v,
            tc.tile_pool(name="ps_r", bufs=1, space="PSUM") as ps_r,
        ):
            mt_t = const_pool.tile([128, FCH * 8], BF16)
            nc.sync.dma_start(mt_t[:], mt_ap[:])
            hb_t = const_pool.tile([8, 1], F32)
            nc.sync.dma_start(hb_t[:], hb_ap[:])
            identb_t = const_pool.tile([128, 128], BF16)
            nc.sync.dma_start(identb_t[:], identb_ap[:])
            id8_t = const_pool.tile([8, 8], F32)
            nc.sync.dma_start(id8_t[:], id8_ap[:])

            acc = acc_pool.tile([128, 2, NT, C], F32)
            nc.vector.memset(acc[:], 0.0)
            ones_red = const_pool.tile([128, 1], F32)
            nc.vector.memset(ones_red[:], 1.0)

            # prefetch all cast-DMAs up front (SWDGE: f32 DRAM -> bf16 SBUF)
            xds = []
            for d in range(n_dma):
                xd = x_pool.tile([128, DMA_BATCH, F], BF16, tag="x")
                nc.gpsimd.dma_start(xd[:], xs_t[d])
                xds.append(xd)

            def xblock(st, fc):
                d, q = divmod(st, DMA_BATCH)
                return xds[d][:, q, fc * 128:(fc + 1) * 128]

            copy_flip = 0
            for g in range(NG):
                ipps = ps_ip.tile([8, GROUP], F32)
                if XBAR:
                    # one xbar-DMA per 128-sample tile transposes all 16
                    # feature chunks at once: xTg[p, e, s] = x[s, e*128+p]
                    xTg = xt_pool.tile([128, FCH, GROUP], BF16, tag="xTg")
                    for q in range(NT):
                        st = g * NT + q
                        d, qq = divmod(st, DMA_BATCH)
                        nc.sync.dma_start(
                            xTg[:, :, q * 128:(q + 1) * 128],
                            xds[d][:, qq, :],
                            transpose=True,
                        )
                    for fc in range(FCH):
                        nc.tensor.matmul(
                            ipps[:],
                            lhsT=mt_t[:, fc * 8:(fc + 1) * 8],
                            rhs=xTg[:, fc, :],
                            start=(fc == 0),
                            stop=(fc == FCH - 1),
                        )
                else:
                    for fcp in range(FCH // FC_PER_COPY):
                        tp = ps_t.tile([128, FC_PER_COPY * GROUP], BF16)
                        for k in range(FC_PER_COPY):
                            for q in range(NT):
                                nc.tensor.transpose(
                                    tp[:, k * GROUP + q * 128:
                                       k * GROUP + (q + 1) * 128],
                                    xblock(g * NT + q, fcp * FC_PER_COPY + k),
                                    identb_t[:],
                                )
                        xT = xt_pool.tile([128, FC_PER_COPY * GROUP], BF16)
                        if COPY_PATTERN[copy_flip % len(COPY_PATTERN)]:
                            nc.scalar.copy(xT[:], tp[:])
                        else:
                            nc.vector.tensor_copy(xT[:], tp[:])
                        copy_flip += 1
                        for k in range(FC_PER_COPY):
                            fc = fcp * FC_PER_COPY + k
                            nc.tensor.matmul(
                                ipps[:],
                                lhsT=mt_t[:, fc * 8:(fc + 1) * 8],
                                rhs=xT[:, k * GROUP:(k + 1) * GROUP],
                                start=(fc == 0),
                                stop=(fc == FCH - 1),
                            )
                # biased scores on ACT: sc[c, s] = ip[c, s] - ||m'_c||^2/2
                sc = sb_pool.tile([8, GROUP], F32, tag="sc")
                nc.scalar.activation(
                    sc[:], ipps[:], mybir.ActivationFunctionType.Identity,
                    bias=hb_t[0:8, 0:1],
                )
                vps = ps_v.tile([128, NT, 8], F32)
                for q in range(NT):
                    nc.tensor.transpose(
                        vps[:, q, :], sc[:, q * 128:(q + 1) * 128], id8_t[:]
                    )
                # vps: [128 samples, q, 8] = 7 biased scores + feature-sum
                mxg = sb_pool.tile([128, NT], F32, tag="mx")
                nc.vector.tensor_reduce(
                    mxg[:], vps[:, :, 0:C], axis=mybir.AxisListType.X,
                    op=AluOpType.max,
                )
                ohg = sb_pool.tile([128, NT, C], F32, tag="oh")
                nc.vector.tensor_tensor(
                    ohg[:], vps[:, :, 0:C], mxg[:].broadcast_to([128, NT, C]),
                    op=AluOpType.is_equal,
                )
                nc.vector.tensor_tensor(
                    acc[:, 0, :, :], acc[:, 0, :, :], ohg[:], op=AluOpType.add
                )
                whg = sb_pool.tile([128, NT, C], F32, tag="whm")
                nc.vector.tensor_tensor(
                    whg[:], ohg[:], vps[:, :, C:C + 1].broadcast_to([128, NT, C]),
                    op=AluOpType.mult,
                )
                nc.vector.tensor_tensor(
                    acc[:, 1, :, :], acc[:, 1, :, :], whg[:], op=AluOpType.add
                )

            rps = ps_r.tile([2 * NT * C, 1], F32)
            nc.tensor.matmul(
                rps[:], lhsT=acc[:].rearrange("p a q c -> p (a q c)"),
                rhs=ones_red[:], start=True, stop=True,
            )
            res_sb = sb_pool.tile([2 * NT * C, 1], F32, tag="res")
            nc.vector.tensor_copy(res_sb[:], rps[:])
            nc.sync.dma_start(out_ap[:], res_sb[:])

    nc.compile()
    return nc


def _get_nc():
    if "nc" not in _cache:
        _cache["nc"] = _build_nc()
    return _cache["nc"]


def _bf16_np():
    import ml_dtypes

    return np.dtype(ml_dtypes.bfloat16)


def _host_inputs(running_mean: np.ndarray):
    mp = running_mean.astype(np.float64) - EPS          # [C, F]
    mt_aug = np.zeros((F, 8), dtype=np.float64)
    mt_aug[:, :C] = mp.T
    mt_aug[:, C] = 1.0
    # SBUF chunk layout: mt[p, c*8+n] = mt_aug[c*128+p, n]
    mt = np.ascontiguousarray(
        mt_aug.reshape(FCH, 128, 8).transpose(1, 0, 2).reshape(128, FCH * 8)
    ).astype(_bf16_np())
    hb = np.zeros((8, 1), dtype=np.float32)
    # bias matches what the PE actually multiplies: the bf16-rounded m'
    mpb = mt_aug[:, :C].astype(_bf16_np()).astype(np.float64)
    hb[:C, 0] = (-0.5 * (mpb * mpb).sum(axis=0)).astype(np.float32)
    identb = np.eye(128).astype(_bf16_np())
    id8 = np.eye(8, dtype=np.float32)
    return mt, hb, identb, id8


def kernel(x: np.ndarray, running_mean: np.ndarray) -> np.ndarray:
    x = np.asarray(x, dtype=np.float32)
    running_mean = np.asarray(running_mean, dtype=np.float32)
    nc = _get_nc()
    mt, hb, identb, id8 = _host_inputs(running_mean)
    in_maps = [
        {
            "xs": np.ascontiguousarray(x[i * BC:(i + 1) * BC]),
            "mt": mt,
            "hb": hb,
            "identb": identb,
            "id8": id8,
        }
        for i in range(NCORES)
    ]
    res = run_bass_kernel_spmd(nc, in_maps, core_ids=list(range(NCORES)))
    counts = np.zeros(C, dtype=np.float32)
    wsums = np.zeros(C, dtype=np.float32)
    for r in res.results:
        p = r["partials"].reshape(2, NT, C)
        counts += p[0].sum(axis=0)
        wsums += p[1].sum(axis=0)
    scalar_mean = wsums / np.maximum(counts * np.float32(F), np.float32(1.0))
    update = (np.float32(MOMENTUM) * scalar_mean)[:, None] + np.float32(
        1.0 - MOMENTUM
    ) * running_mean
    out = np.where((counts > COUNT_THRESH)[:, None], update, running_mean)
    return out.astype(np.float32)
